# revision 30
# baseline (speedup 1.0000x reference)
"""Two-layer single-head GAT on 8 TRN2 NeuronCores (Bass/Tile, SPMD).

Strategy (graph/data parallel, dst-sharded):
  - Nodes are sharded contiguously across the 8 cores (12544 per core, 100352
    padded total). Each core owns the attention aggregation for its dst nodes.
  - Per layer, each core computes its shard of an augmented node table
      row(v) = [h(v) (64), h@a_src, h@a_dst, is_real(v), 0] (68 f32, 272B)
    with one PE matmul per 128 nodes, then shards are AllGathered (in two
    halves, overlapped with the shard build) so every core holds the full
    table in DRAM.
  - Edge phase: the core's dst nodes are sorted by degree and tiled 128 nodes
    per tile (dst node = partition, K_t edge slots along free dim; K_t is the
    cross-core max so the SPMD program is identical on all cores). Src rows
    are fetched with one indirect DMA per slot column (128 rows each); the
    self-loop edge is forced into slot 0 of every dst so the gathered row
    doubles as the source of s_dst (no separate per-tile s_dst gather). Pad
    slots point at the all-zero phantom row and contribute nothing.
      e = exp(leaky_relu(s_src + s_dst)) = max(exp(x), exp(0.2 x))
    Gathered rows are scaled by e and segment-reduced with one strided DVE
    reduction per tile; column 66 of the reduction is the softmax denominator.
    Rows are normalized, biased, (relu'd) and scattered to the output shard by
    local row id (pad nodes carry an out-of-bounds id and are dropped).
  - The layer-2 node table is built INSIDE edge phase 1: each tile's output
    rows (in degree-rank order) are transposed on the PE and projected through
    W2aug immediately, stored affine in rank order. Layer-2 gather metadata is
    rank-indexed so no scatter is needed for the table; the phantom row is the
    last rank of core 7 (a zero-degree pad node, masked to zero).

All metadata (gather/scatter indices) is precomputed on the host from
edge_index only and fed as an int32 input, so one compiled SPMD program
serves all cores.
"""
import os
import numpy as np
from contextlib import ExitStack

import concourse.bass as bass
import concourse.tile as tile
from concourse import mybir
from concourse.bass_utils import run_bass_kernel_spmd
from concourse.masks import make_identity
from concourse.vector_clock import ScopedClock

# ---------------------------------------------------------------- constants
N = 100_000
D_IN, D_HID, D_OUT = 128, 64, 64
NEG_SLOPE = 0.2
N_CORES = 8
SHARD = 12_544            # 98 * 128
NPAD = SHARD * N_CORES    # 100352
NT = SHARD // 128         # 98 node tiles per core
DT = 68                   # table row width (64 feat, s_src, s_dst, one, pad)
PHANTOM = NPAD - 1        # guaranteed all-zero table row (both layers)
OOB_ROW = 1 << 29
WCAP = 96                 # max edge slots (sum of K_t) per gather group
GCAP = 12                 # max tiles per group

F32 = mybir.dt.float32
I32 = mybir.dt.int32

_noop_ctr = [0]


# ------------------------------------------------------- tile-drain patch
def _patched_drain_and_barrier(self, tick_clock, wait_clock):
    """walrus codegen in this container refuses any sem wait on the Drain
    instruction; emit the kernel-tail waits as standalone single-wait NOPs
    on the sync engine instead, then an untainted drain."""
    probe = self.nc.sync.nop(nofuse=True)
    wait_clock.add_sem_waits(probe.ins, ScopedClock({None: tick_clock.global_clock}))
    si = probe.ins.sync_info
    waits = list(si.on_wait) if si is not None and si.on_wait else []
    if si is not None:
        probe.ins.sync_info = mybir.SyncInfo(
            on_wait=waits[:1], on_update=list(si.on_update or [])
        )
    for w in waits[1:]:
        nop = self.nc.sync.nop(nofuse=True)
        nop.ins.sync_info = mybir.SyncInfo(on_wait=[w], on_update=[])
    self.nc.sync.drain()
    self.nc.all_engine_barrier()
    popped = self.nc._tile_sem_poison_stack.pop()
    assert popped is self._sem_poison
    self.nc.clear_and_free_semaphores(list(self.sems.allocated().values()))
    self.nc.all_engine_barrier()


tile.TileContext._drain_and_barrier = _patched_drain_and_barrier


def _legalize_waits(nc, keep=1):
    """walrus codegen allows very few sem waits per ISA instruction (1 for
    DMAs/matmuls, 0 for Drain). Hoist excess waits onto single-wait NoOps
    on the same engine immediately before the instruction — engine program
    order preserves the blocking semantics exactly."""
    for bb in nc.main_func.blocks:
        insts = bb.instructions
        new_list = []
        changed = False
        for ins in insts:
            si = getattr(ins, "sync_info", None)
            waits = list(si.on_wait) if si is not None and si.on_wait else []
            k = 0 if ins.opcode in ("Drain",) else keep
            if len(waits) > k:
                changed = True
                for w in waits[k:]:
                    nop = mybir.InstNoOp(name=f"waitnop-{_noop_ctr[0]}", ins=[], outs=[])
                    _noop_ctr[0] += 1
                    nop.engine = ins.engine
                    nop.sync_info = mybir.SyncInfo(on_wait=[w], on_update=[])
                    new_list.append(nop)
                ins.sync_info = mybir.SyncInfo(
                    on_wait=waits[:k], on_update=list(si.on_update or [])
                )
            new_list.append(ins)
        if changed:
            insts[:] = new_list


# ------------------------------------------------------------- host prep
def _build_metadata(edge_index):
    """Host-side graph preprocessing from edge_index only.

    Self-loops are prepended so each dst's slot 0 is its own table row (the
    source of s_dst). Layer 1 gathers by global node id; layer 2 gathers by
    global RANK (the degree-sort order the layer-2 table is stored in).

    Returns (metas, groups): metas[c] is the flat int32 metadata (identical
    length per core); groups is the uniform group structure [(Ks, W), ...].
    Per group the metadata block is [128, 2*W + G] int32:
      cols [0, W)        layer-1 gather rows (global ids, PHANTOM for pads)
      cols [W, 2W)       layer-2 gather rows (global ranks, PHANTOM for pads)
      cols [2W, 2W+G)    scatter rows (local out row, OOB_ROW for pad nodes)
    """
    ei = np.asarray(edge_index)
    # self-loops FIRST so the stable dst-sort puts them at slot 0 of each dst
    src = np.concatenate([np.arange(N, dtype=np.int64), ei[0]]).astype(np.int64)
    dst = np.concatenate([np.arange(N, dtype=np.int64), ei[1]]).astype(np.int64)
    PART = SHARD // 7

    def _grow(v):
        """node/rank id -> row in the part-major full-table layout."""
        c, r = v // SHARD, v % SHARD
        return (r // PART) * (N_CORES * PART) + c * PART + (r % PART)

    per_core = []
    for c in range(N_CORES):
        lo, hi = c * SHARD, (c + 1) * SHARD
        m = (dst >= lo) & (dst < hi)
        s_c, d_c = src[m], dst[m] - lo
        order = np.argsort(d_c, kind="stable")
        s_c, d_c = s_c[order], d_c[order]
        deg = np.bincount(d_c, minlength=SHARD)
        starts = np.concatenate([[0], np.cumsum(deg)[:-1]])
        perm = np.argsort(-deg, kind="stable")
        # global rank of each global node id (for layer-2 table addressing)
        per_core.append((s_c, deg, starts, perm))

    # rank lookup: grank[global id] = core*SHARD + rank within core
    grank = np.empty(NPAD, dtype=np.int64)
    for c in range(N_CORES):
        _, _, _, perm = per_core[c]
        rank = np.empty(SHARD, dtype=np.int64)
        rank[perm] = np.arange(SHARD)
        grank[c * SHARD:(c + 1) * SHARD] = c * SHARD + rank

    Ks = np.zeros(NT, dtype=np.int64)
    for c in range(N_CORES):
        _, deg, _, perm = per_core[c]
        Ks = np.maximum(Ks, deg[perm].reshape(NT, 128).max(axis=1))
    Ks = np.maximum(Ks, 1)

    groups = []
    cur, curW = [], 0
    for t in range(NT):
        if cur and (curW + Ks[t] > WCAP or len(cur) >= GCAP):
            groups.append(cur)
            cur, curW = [], 0
        cur.append(t)
        curW += Ks[t]
    if cur:
        groups.append(cur)

    metas = []
    for c in range(N_CORES):
        s_c, deg, starts, perm = per_core[c]
        blocks = []
        for g in groups:
            cols1, cols2 = [], []
            outr = np.empty((128, len(g)), dtype=np.int64)
            for gi, t in enumerate(g):
                K = int(Ks[t])
                vs = perm[t * 128:(t + 1) * 128]
                dv = deg[vs]
                st = starts[vs]
                ar = np.arange(K)[None, :]
                valid = ar < dv[:, None]
                pos = np.minimum(st[:, None] + ar, max(len(s_c) - 1, 0))
                gsrc = np.where(valid, s_c[pos] if len(s_c) else PHANTOM, PHANTOM)
                cols1.append(_grow(gsrc))
                cols2.append(_grow(np.where(valid, grank[gsrc], PHANTOM)))
                real = dv > 0
                outr[:, gi] = np.where(real, vs, OOB_ROW)
            block = np.concatenate(cols1 + cols2 + [outr], axis=1)
            blocks.append(block.ravel())
        metas.append(np.concatenate(blocks).astype(np.int32))

    return metas, [(np.array([int(Ks[t]) for t in g]), int(sum(Ks[t] for t in g)))
                   for g in groups]


# ---------------------------------------------------------- device program
def _build_program(groups, meta_len):
    nc = bass.Bass(num_devices=N_CORES)

    xT_p = nc.declare_dram_parameter("xT", [D_IN, SHARD], F32, isOutput=False)
    meta_p = nc.declare_dram_parameter("meta", [meta_len], I32, isOutput=False)
    w1_p = nc.declare_dram_parameter("w1aug", [D_IN, DT], F32, isOutput=False)
    w2_p = nc.declare_dram_parameter("w2aug", [D_HID, DT], F32, isOutput=False)
    b1_p = nc.declare_dram_parameter("b1", [128, 64], F32, isOutput=False)
    b2_p = nc.declare_dram_parameter("b2", [128, 64], F32, isOutput=False)
    oc_p = nc.declare_dram_parameter("onecol", [SHARD], F32, isOutput=False)
    ocp_p = nc.declare_dram_parameter("onecolp", [SHARD], F32, isOutput=False)
    emb_p = nc.declare_dram_parameter("emb", [SHARD, D_HID], F32, isOutput=True)
    out_p = nc.declare_dram_parameter("out", [SHARD, D_OUT], F32, isOutput=True)

    tab_shard = [nc.dram_tensor(f"tab{l}_shard", [SHARD, DT], F32) for l in (1, 2)]
    tab_full = [nc.dram_tensor(f"tab{l}_full", [NPAD, DT], F32) for l in (1, 2)]

    NPART = 7                    # AllGather pipeline depth (NT = 7 * 14)
    PART = SHARD // NPART        # rows per part (14 tiles)
    TPP = NT // NPART            # tiles per part

    with tile.TileContext(nc) as tc, ExitStack() as ctx:
        cpool = ctx.enter_context(tc.tile_pool(name="const", bufs=1))
        ident = cpool.tile([128, 128], F32)
        make_identity(nc, ident[:])
        w1_sb = cpool.tile([D_IN, DT], F32)
        nc.sync.dma_start(out=w1_sb[:], in_=w1_p[:])
        w2_sb = cpool.tile([D_HID, DT], F32)
        nc.sync.dma_start(out=w2_sb[:], in_=w2_p[:])
        oc_sb = cpool.tile([128, NT], F32)
        nc.sync.dma_start(out=oc_sb[:], in_=oc_p[:].rearrange("(t p) -> p t", p=128))
        ocp_sb = cpool.tile([128, NT], F32)
        nc.sync.dma_start(out=ocp_sb[:], in_=ocp_p[:].rearrange("(t p) -> p t", p=128))
        b1_sb = cpool.tile([128, 64], F32)
        nc.sync.dma_start(out=b1_sb[:], in_=b1_p[:])
        b2_sb = cpool.tile([128, 64], F32)
        nc.sync.dma_start(out=b2_sb[:], in_=b2_p[:])

        bc_reg = nc.gpsimd.alloc_register()
        nc.gpsimd.reg_mov(bc_reg, SHARD - 1)

        def allgather_part(shard_dram, full_dram, h):
            # full table is laid out part-major: rows [h*8*PART, (h+1)*8*PART)
            # hold part h of every core's shard, concatenated by core — so the
            # collective output is contiguous. Host gather indices use the
            # same layout (see _grow).
            lo, hi = h * PART, (h + 1) * PART
            base = h * N_CORES * PART
            nc.gpsimd.collective_compute(
                "AllGather",
                mybir.AluOpType.bypass,
                replica_groups=[list(range(N_CORES))],
                ins=[shard_dram[lo:hi, :]],
                outs=[full_dram[base:base + N_CORES * PART, :]],
            )
            # make Pool observe the collective once (DMA insts get 1 wait slot)
            fun = cpool.tile([1, DT], F32, tag=f"funnel{h}")
            nc.gpsimd.dma_start(out=fun[:], in_=full_dram[base:base + 1, :])

        # ---------------- layer-1 table build (id order) ----------------
        def build_table1():
            with tc.tile_pool(name="tbl", bufs=8) as tp, \
                 tc.tile_pool(name="tblp", bufs=6, space="PSUM") as pp, \
                 tc.tile_pool(name="tblbig", bufs=1) as bigp:
                big = bigp.tile([128, NT, DT], F32)
                for t in range(NT):
                    xT = tp.tile([D_IN, 128], F32, tag="xT")
                    nc.sync.dma_start(out=xT[:], in_=xT_p[:, t * 128:(t + 1) * 128])
                    psh = pp.tile([128, DT], F32, tag="psh")
                    nc.tensor.matmul(out=psh[:], lhsT=xT[:], rhs=w1_sb[:], start=True, stop=True)
                    nc.vector.tensor_copy(out=big[:, t, :], in_=psh[:])
                    nc.vector.tensor_copy(out=big[:, t, 66:67], in_=oc_sb[:, t:t + 1])
                    if (t + 1) % TPP == 0:
                        h = (t + 1) // TPP - 1
                        lo, hi = h * TPP, (h + 1) * TPP
                        nc.sync.dma_start(
                            out=tab_shard[0][lo * 128:hi * 128, :].rearrange(
                                "(t p) d -> p t d", p=128),
                            in_=big[:, lo:hi, :],
                        )
                        allgather_part(tab_shard[0], tab_full[0], h)

        # ---------------- edge phase for one layer ----------------
        def edge_phase(layer, full_dram, bias_sb, relu, dst_dram, big2):
            with tc.tile_pool(name="edg", bufs=3) as ep, \
                 tc.tile_pool(name="edg2", bufs=3) as ep2, \
                 tc.tile_pool(name="edgp", bufs=3, space="PSUM") as pp:
                moff = 0
                tglob = 0
                half_state = [0]
                pending = None  # deferred scatters: (meta_t, o3, W, G)

                def flush_pending():
                    if pending is None:
                        return
                    pm, po3, pW, pG = pending
                    for gi in range(pG):
                        nc.gpsimd.indirect_dma_start(
                            out=dst_dram[:],
                            out_offset=bass.IndirectOffsetOnAxis(
                                ap=pm[:, 2 * pW + gi:2 * pW + gi + 1], axis=0
                            ),
                            in_=po3[:, gi, :],
                            in_offset=None,
                            bounds_check=bc_reg,
                            oob_is_err=False,
                        )

                for Ks_g, W in groups:
                    G = len(Ks_g)
                    mw = 2 * W + G
                    meta_t = ep.tile([128, mw], I32, tag="meta")
                    nc.sync.dma_start(
                        out=meta_t[:],
                        in_=meta_p[moff:moff + 128 * mw].rearrange("(p w) -> p w", p=128),
                    )
                    moff += 128 * mw
                    mcol = 0 if layer == 1 else W

                    gat = ep.tile([128, W * DT], F32, tag="gat")
                    g3 = gat[:].rearrange("p (w d) -> p w d", d=DT)
                    for w in range(W):
                        nc.gpsimd.indirect_dma_start(
                            out=g3[:, w, :],
                            out_offset=None,
                            in_=full_dram[:],
                            in_offset=bass.IndirectOffsetOnAxis(
                                ap=meta_t[:, mcol + w:mcol + w + 1], axis=0
                            ),
                        )
                    # previous group's scatters issue here so the pool engine
                    # never stalls waiting for this group's DVE chain
                    flush_pending()
                    pending = None

                    # slot 0 of each tile is the self-loop row: col 65 = s_dst
                    lg = ep.tile([128, W], F32, tag="lg")
                    o = 0
                    for gi in range(G):
                        K = int(Ks_g[gi])
                        nc.vector.tensor_tensor(
                            out=lg[:, o:o + K],
                            in0=g3[:, o:o + K, 64],
                            in1=g3[:, o, 65:66].to_broadcast([128, K]),
                            op=mybir.AluOpType.add,
                        )
                        o += K
                    # e = exp(leaky_relu(lg)) = max(exp(lg), exp(0.2*lg))
                    e1 = ep.tile([128, W], F32, tag="e1")
                    nc.scalar.activation(out=e1[:], in_=lg[:], func=mybir.ActivationFunctionType.Exp)
                    e2 = ep.tile([128, W], F32, tag="e2")
                    nc.scalar.activation(out=e2[:], in_=lg[:], func=mybir.ActivationFunctionType.Exp, scale=NEG_SLOPE)
                    nc.vector.tensor_tensor(out=e1[:], in0=e1[:], in1=e2[:], op=mybir.AluOpType.max)

                    nc.vector.tensor_tensor(
                        out=g3[:, :, :],
                        in0=g3[:, :, :],
                        in1=e1[:].to_broadcast([128, W, DT]),
                        op=mybir.AluOpType.mult,
                    )

                    red = ep2.tile([128, G * DT], F32, tag="red")
                    r3 = red[:].rearrange("p (g d) -> p g d", d=DT)
                    o = 0
                    for gi in range(G):
                        K = int(Ks_g[gi])
                        nc.vector.tensor_reduce(
                            out=r3[:, gi, :],
                            in_=g3[:, o:o + K, :].rearrange("p k d -> p d k"),
                            axis=mybir.AxisListType.X,
                            op=mybir.AluOpType.add,
                        )
                        o += K
                    # epsilon so zero-degree pad rows give 0 * huge = 0, not NaN
                    den = ep2.tile([128, G], F32, tag="den")
                    nc.vector.tensor_scalar_add(den[:], r3[:, :, 66], 1e-16)
                    rec = ep2.tile([128, G], F32, tag="rec")
                    nc.vector.reciprocal(out=rec[:], in_=den[:])
                    outt = ep2.tile([128, G * 64], F32, tag="outt")
                    o3 = outt[:].rearrange("p (g d) -> p g d", d=64)
                    nc.vector.tensor_tensor(
                        out=o3[:, :, :],
                        in0=r3[:, :, 0:64],
                        in1=rec[:].to_broadcast([128, G, 64]),
                        op=mybir.AluOpType.mult,
                    )
                    for gi in range(G):
                        nc.vector.tensor_tensor(
                            out=o3[:, gi, :], in0=o3[:, gi, :], in1=bias_sb[:],
                            op=mybir.AluOpType.add,
                        )
                    if relu:
                        nc.vector.tensor_scalar_max(outt[:], outt[:], 0.0)

                    pending = (meta_t, o3, W, G)

                    if big2 is not None:
                        # fused layer-2 table build, rank order (affine)
                        for gi in range(G):
                            t = tglob + gi
                            o3m = ep2.tile([128, 64], F32, tag="o3m")
                            nc.vector.tensor_tensor(
                                out=o3m[:], in0=o3[:, gi, :],
                                in1=ocp_sb[:, t:t + 1].to_broadcast([128, 64]),
                                op=mybir.AluOpType.mult,
                            )
                            pse = pp.tile([64, 128], F32, tag="pse")
                            nc.tensor.transpose(out=pse[:], in_=o3m[:], identity=ident[:])
                            eT = ep2.tile([64, 128], F32, tag="eT")
                            nc.vector.tensor_copy(out=eT[:], in_=pse[:])
                            ps2 = pp.tile([128, DT], F32, tag="ps2")
                            nc.tensor.matmul(out=ps2[:], lhsT=eT[:], rhs=w2_sb[:], start=True, stop=True)
                            nc.vector.tensor_copy(out=big2[:, t, :], in_=ps2[:])
                            nc.vector.tensor_copy(out=big2[:, t, 66:67], in_=ocp_sb[:, t:t + 1])
                        done = tglob + G
                        while (half_state[0] + 1) * TPP <= done:
                            h = half_state[0]
                            lo, hi = h * TPP, (h + 1) * TPP
                            nc.sync.dma_start(
                                out=tab_shard[1][lo * 128:hi * 128, :].rearrange(
                                    "(t p) d -> p t d", p=128),
                                in_=big2[:, lo:hi, :],
                            )
                            allgather_part(tab_shard[1], tab_full[1], h)
                            half_state[0] += 1
                    tglob += G
                flush_pending()

        build_table1()
        with tc.tile_pool(name="big2", bufs=1) as big2p:
            big2 = big2p.tile([128, NT, DT], F32)
            edge_phase(1, tab_full[0], b1_sb, True, emb_p, big2)
            edge_phase(2, tab_full[1], b2_sb, False, out_p, None)

    _legalize_waits(nc)
    return nc


def _ensure_ntff_hook():
    """Best-effort: register the NTFF profile hook that bass_utils expects
    under axon (this agent image's antenv lacks axon_hooks)."""
    try:
        from antenv.axon_hooks import get_axon_ntff_profile_hook  # noqa: F401
        return True
    except ImportError:
        pass
    try:
        import sys
        import types
        import antenv
        from trn_agent_boot.trn_boot import _ntff_profile_via_ctypes
        hook = _ntff_profile_via_ctypes("/opt/axon/libaxon_pjrt.so")
        m = types.ModuleType("antenv.axon_hooks")
        m.get_axon_ntff_profile_hook = lambda: hook
        m.set_axon_ntff_profile_hook = lambda h: None
        sys.modules["antenv.axon_hooks"] = m
        antenv.axon_hooks = m
        return True
    except Exception:
        return False


# ----------------------------------------------------------------- driver
def kernel(x, edge_index, W1, a_src1, a_dst1, b1, W2, a_src2, a_dst2, b2):
    x = np.asarray(x, dtype=np.float32)
    W1 = np.asarray(W1, dtype=np.float32)
    W2 = np.asarray(W2, dtype=np.float32)
    a_src1 = np.asarray(a_src1, dtype=np.float32)
    a_dst1 = np.asarray(a_dst1, dtype=np.float32)
    a_src2 = np.asarray(a_src2, dtype=np.float32)
    a_dst2 = np.asarray(a_dst2, dtype=np.float32)
    b1 = np.asarray(b1, dtype=np.float32)
    b2 = np.asarray(b2, dtype=np.float32)

    metas, groups = _build_metadata(edge_index)
    nc = _build_program(groups, len(metas[0]))

    w1aug = np.zeros((D_IN, DT), dtype=np.float32)
    w1aug[:, :D_HID] = W1
    w1aug[:, 64] = W1 @ a_src1
    w1aug[:, 65] = W1 @ a_dst1
    w2aug = np.zeros((D_HID, DT), dtype=np.float32)
    w2aug[:, :D_OUT] = W2
    w2aug[:, 64] = W2 @ a_src2
    w2aug[:, 65] = W2 @ a_dst2

    x_pad = np.zeros((NPAD, D_IN), dtype=np.float32)
    x_pad[:N] = x

    # recompute the degree-sort perms to build the rank-ordered one-mask
    ei = np.asarray(edge_index)
    src = np.concatenate([np.arange(N, dtype=np.int64), ei[0]]).astype(np.int64)
    dst = np.concatenate([np.arange(N, dtype=np.int64), ei[1]]).astype(np.int64)

    in_maps = []
    for c in range(N_CORES):
        oc = np.zeros(SHARD, dtype=np.float32)
        n_real = min(max(N - c * SHARD, 0), SHARD)
        oc[:n_real] = 1.0
        lo, hi = c * SHARD, (c + 1) * SHARD
        m = (dst >= lo) & (dst < hi)
        deg = np.bincount(dst[m] - lo, minlength=SHARD)
        perm = np.argsort(-deg, kind="stable")
        ocp = oc[perm]
        in_maps.append({
            "xT": np.ascontiguousarray(x_pad[c * SHARD:(c + 1) * SHARD].T),
            "meta": metas[c],
            "w1aug": w1aug,
            "w2aug": w2aug,
            "b1": np.tile(b1, (128, 1)),
            "b2": np.tile(b2, (128, 1)),
            "onecol": oc,
            "onecolp": ocp,
        })

    trace = bool(os.environ.get("GAT_KERNEL_TRACE"))
    if trace:
        trace = _ensure_ntff_hook()
    res = run_bass_kernel_spmd(nc, in_maps, list(range(N_CORES)), trace=trace)
    if trace and res.exec_time_ns is not None:
        print(f"HW exec time: {res.exec_time_ns} ns")

    emb = np.concatenate([res.results[c]["emb"] for c in range(N_CORES)], axis=0)[:N]
    out = np.concatenate([res.results[c]["out"] for c in range(N_CORES)], axis=0)[:N]
    return (emb, out)


# revision 32
# speedup vs baseline: 1.0057x; 1.0057x over previous
"""Two-layer single-head GAT on 8 TRN2 NeuronCores (Bass/Tile, SPMD).

Strategy (graph/data parallel, dst-sharded):
  - Nodes are sharded contiguously across the 8 cores (12544 per core, 100352
    padded total). Each core owns the attention aggregation for its dst nodes.
  - Per layer, each core computes its shard of an augmented node table
      row(v) = [h(v) (64), h@a_src, h@a_dst, is_real(v), 0] (68 f32, 272B)
    with one PE matmul per 128 nodes, then shards are AllGathered (in two
    halves, overlapped with the shard build) so every core holds the full
    table in DRAM.
  - Edge phase: the core's dst nodes are sorted by degree and tiled 128 nodes
    per tile (dst node = partition, K_t edge slots along free dim; K_t is the
    cross-core max so the SPMD program is identical on all cores). Src rows
    are fetched with one indirect DMA per slot column (128 rows each); the
    self-loop edge is forced into slot 0 of every dst so the gathered row
    doubles as the source of s_dst (no separate per-tile s_dst gather). Pad
    slots point at the all-zero phantom row and contribute nothing.
      e = exp(leaky_relu(s_src + s_dst)) = max(exp(x), exp(0.2 x))
    Gathered rows are scaled by e and segment-reduced with one strided DVE
    reduction per tile; column 66 of the reduction is the softmax denominator.
    Rows are normalized, biased, (relu'd) and scattered to the output shard by
    local row id (pad nodes carry an out-of-bounds id and are dropped).
  - The layer-2 node table is built INSIDE edge phase 1: each tile's output
    rows (in degree-rank order) are transposed on the PE and projected through
    W2aug immediately, stored affine in rank order. Layer-2 gather metadata is
    rank-indexed so no scatter is needed for the table; the phantom row is the
    last rank of core 7 (a zero-degree pad node, masked to zero).

All metadata (gather/scatter indices) is precomputed on the host from
edge_index only and fed as an int32 input, so one compiled SPMD program
serves all cores.
"""
import os
import numpy as np
from contextlib import ExitStack

import concourse.bass as bass
import concourse.tile as tile
from concourse import mybir
from concourse.bass_utils import run_bass_kernel_spmd
from concourse.masks import make_identity
from concourse.vector_clock import ScopedClock

# ---------------------------------------------------------------- constants
N = 100_000
D_IN, D_HID, D_OUT = 128, 64, 64
NEG_SLOPE = 0.2
N_CORES = 8
SHARD = 12_544            # 98 * 128
NPAD = SHARD * N_CORES    # 100352
NT = SHARD // 128         # 98 node tiles per core
DT = 68                   # table row width (64 feat, s_src, s_dst, one, pad)
PHANTOM = NPAD - 1        # guaranteed all-zero table row (both layers)
OOB_ROW = 1 << 29
WCAP = 96                 # max edge slots (sum of K_t) per gather group
GCAP = 12                 # max tiles per group

F32 = mybir.dt.float32
I32 = mybir.dt.int32

_noop_ctr = [0]


# ------------------------------------------------------- tile-drain patch
def _patched_drain_and_barrier(self, tick_clock, wait_clock):
    """walrus codegen in this container refuses any sem wait on the Drain
    instruction; emit the kernel-tail waits as standalone single-wait NOPs
    on the sync engine instead, then an untainted drain."""
    probe = self.nc.sync.nop(nofuse=True)
    wait_clock.add_sem_waits(probe.ins, ScopedClock({None: tick_clock.global_clock}))
    si = probe.ins.sync_info
    waits = list(si.on_wait) if si is not None and si.on_wait else []
    if si is not None:
        probe.ins.sync_info = mybir.SyncInfo(
            on_wait=waits[:1], on_update=list(si.on_update or [])
        )
    for w in waits[1:]:
        nop = self.nc.sync.nop(nofuse=True)
        nop.ins.sync_info = mybir.SyncInfo(on_wait=[w], on_update=[])
    self.nc.sync.drain()
    self.nc.all_engine_barrier()
    popped = self.nc._tile_sem_poison_stack.pop()
    assert popped is self._sem_poison
    self.nc.clear_and_free_semaphores(list(self.sems.allocated().values()))
    self.nc.all_engine_barrier()


tile.TileContext._drain_and_barrier = _patched_drain_and_barrier


def _legalize_waits(nc, keep=1):
    """walrus codegen allows very few sem waits per ISA instruction (1 for
    DMAs/matmuls, 0 for Drain). Hoist excess waits onto single-wait NoOps
    on the same engine immediately before the instruction — engine program
    order preserves the blocking semantics exactly."""
    for bb in nc.main_func.blocks:
        insts = bb.instructions
        new_list = []
        changed = False
        for ins in insts:
            si = getattr(ins, "sync_info", None)
            waits = list(si.on_wait) if si is not None and si.on_wait else []
            k = 0 if ins.opcode in ("Drain",) else keep
            if len(waits) > k:
                changed = True
                for w in waits[k:]:
                    nop = mybir.InstNoOp(name=f"waitnop-{_noop_ctr[0]}", ins=[], outs=[])
                    _noop_ctr[0] += 1
                    nop.engine = ins.engine
                    nop.sync_info = mybir.SyncInfo(on_wait=[w], on_update=[])
                    new_list.append(nop)
                ins.sync_info = mybir.SyncInfo(
                    on_wait=waits[:k], on_update=list(si.on_update or [])
                )
            new_list.append(ins)
        if changed:
            insts[:] = new_list


# ------------------------------------------------------------- host prep
def _build_metadata(edge_index):
    """Host-side graph preprocessing from edge_index only.

    Self-loops are prepended so each dst's slot 0 is its own table row (the
    source of s_dst). Layer 1 gathers by global node id; layer 2 gathers by
    global RANK (the degree-sort order the layer-2 table is stored in).

    Returns (metas, groups): metas[c] is the flat int32 metadata (identical
    length per core); groups is the uniform group structure [(Ks, W), ...].
    Per group the metadata block is [128, 2*W + G] int32:
      cols [0, W)        layer-1 gather rows (global ids, PHANTOM for pads)
      cols [W, 2W)       layer-2 gather rows (global ranks, PHANTOM for pads)
      cols [2W, 2W+G)    scatter rows (local out row, OOB_ROW for pad nodes)
    """
    ei = np.asarray(edge_index)
    # self-loops FIRST so the stable dst-sort puts them at slot 0 of each dst
    src = np.concatenate([np.arange(N, dtype=np.int64), ei[0]]).astype(np.int64)
    dst = np.concatenate([np.arange(N, dtype=np.int64), ei[1]]).astype(np.int64)
    PART = SHARD // 2

    def _grow(v):
        """node/rank id -> row in the part-major full-table layout."""
        c, r = v // SHARD, v % SHARD
        return (r // PART) * (N_CORES * PART) + c * PART + (r % PART)

    per_core = []
    for c in range(N_CORES):
        lo, hi = c * SHARD, (c + 1) * SHARD
        m = (dst >= lo) & (dst < hi)
        s_c, d_c = src[m], dst[m] - lo
        order = np.argsort(d_c, kind="stable")
        s_c, d_c = s_c[order], d_c[order]
        deg = np.bincount(d_c, minlength=SHARD)
        starts = np.concatenate([[0], np.cumsum(deg)[:-1]])
        perm = np.argsort(-deg, kind="stable")
        # global rank of each global node id (for layer-2 table addressing)
        per_core.append((s_c, deg, starts, perm))

    # rank lookup: grank[global id] = core*SHARD + rank within core
    grank = np.empty(NPAD, dtype=np.int64)
    for c in range(N_CORES):
        _, _, _, perm = per_core[c]
        rank = np.empty(SHARD, dtype=np.int64)
        rank[perm] = np.arange(SHARD)
        grank[c * SHARD:(c + 1) * SHARD] = c * SHARD + rank

    Ks = np.zeros(NT, dtype=np.int64)
    for c in range(N_CORES):
        _, deg, _, perm = per_core[c]
        Ks = np.maximum(Ks, deg[perm].reshape(NT, 128).max(axis=1))
    Ks = np.maximum(Ks, 1)

    groups = []
    cur, curW = [], 0
    for t in range(NT):
        if cur and (curW + Ks[t] > WCAP or len(cur) >= GCAP):
            groups.append(cur)
            cur, curW = [], 0
        cur.append(t)
        curW += Ks[t]
    if cur:
        groups.append(cur)

    metas = []
    for c in range(N_CORES):
        s_c, deg, starts, perm = per_core[c]
        blocks = []
        for g in groups:
            cols1, cols2 = [], []
            outr = np.empty((128, len(g)), dtype=np.int64)
            for gi, t in enumerate(g):
                K = int(Ks[t])
                vs = perm[t * 128:(t + 1) * 128]
                dv = deg[vs]
                st = starts[vs]
                ar = np.arange(K)[None, :]
                valid = ar < dv[:, None]
                pos = np.minimum(st[:, None] + ar, max(len(s_c) - 1, 0))
                gsrc = np.where(valid, s_c[pos] if len(s_c) else PHANTOM, PHANTOM)
                cols1.append(_grow(gsrc))
                cols2.append(_grow(np.where(valid, grank[gsrc], PHANTOM)))
                real = dv > 0
                outr[:, gi] = np.where(real, vs, OOB_ROW)
            block = np.concatenate(cols1 + cols2 + [outr], axis=1)
            blocks.append(block.ravel())
        metas.append(np.concatenate(blocks).astype(np.int32))

    return metas, [(np.array([int(Ks[t]) for t in g]), int(sum(Ks[t] for t in g)))
                   for g in groups]


# ---------------------------------------------------------- device program
def _build_program(groups, meta_len):
    nc = bass.Bass(num_devices=N_CORES)

    xT_p = nc.declare_dram_parameter("xT", [D_IN, SHARD], F32, isOutput=False)
    meta_p = nc.declare_dram_parameter("meta", [meta_len], I32, isOutput=False)
    w1_p = nc.declare_dram_parameter("w1aug", [D_IN, DT], F32, isOutput=False)
    w2_p = nc.declare_dram_parameter("w2aug", [D_HID, DT], F32, isOutput=False)
    b1_p = nc.declare_dram_parameter("b1", [128, 64], F32, isOutput=False)
    b2_p = nc.declare_dram_parameter("b2", [128, 64], F32, isOutput=False)
    oc_p = nc.declare_dram_parameter("onecol", [SHARD], F32, isOutput=False)
    ocp_p = nc.declare_dram_parameter("onecolp", [SHARD], F32, isOutput=False)
    emb_p = nc.declare_dram_parameter("emb", [SHARD, D_HID], F32, isOutput=True)
    out_p = nc.declare_dram_parameter("out", [SHARD, D_OUT], F32, isOutput=True)

    tab_shard = [nc.dram_tensor(f"tab{l}_shard", [SHARD, DT], F32) for l in (1, 2)]
    tab_full = [nc.dram_tensor(f"tab{l}_full", [NPAD, DT], F32) for l in (1, 2)]

    NPART = 2                    # AllGather pipeline depth (divides NT)
    PART = SHARD // NPART        # rows per part
    TPP = NT // NPART            # tiles per part

    with tile.TileContext(nc) as tc, ExitStack() as ctx:
        cpool = ctx.enter_context(tc.tile_pool(name="const", bufs=1))
        ident = cpool.tile([128, 128], F32)
        make_identity(nc, ident[:])
        w1_sb = cpool.tile([D_IN, DT], F32)
        nc.sync.dma_start(out=w1_sb[:], in_=w1_p[:])
        w2_sb = cpool.tile([D_HID, DT], F32)
        nc.sync.dma_start(out=w2_sb[:], in_=w2_p[:])
        oc_sb = cpool.tile([128, NT], F32)
        nc.sync.dma_start(out=oc_sb[:], in_=oc_p[:].rearrange("(t p) -> p t", p=128))
        ocp_sb = cpool.tile([128, NT], F32)
        nc.sync.dma_start(out=ocp_sb[:], in_=ocp_p[:].rearrange("(t p) -> p t", p=128))
        b1_sb = cpool.tile([128, 64], F32)
        nc.sync.dma_start(out=b1_sb[:], in_=b1_p[:])
        b2_sb = cpool.tile([128, 64], F32)
        nc.sync.dma_start(out=b2_sb[:], in_=b2_p[:])

        bc_reg = nc.gpsimd.alloc_register()
        nc.gpsimd.reg_mov(bc_reg, SHARD - 1)

        def allgather_part(shard_dram, full_dram, h):
            # full table is laid out part-major: rows [h*8*PART, (h+1)*8*PART)
            # hold part h of every core's shard, concatenated by core — so the
            # collective output is contiguous. Host gather indices use the
            # same layout (see _grow).
            lo, hi = h * PART, (h + 1) * PART
            base = h * N_CORES * PART
            nc.gpsimd.collective_compute(
                "AllGather",
                mybir.AluOpType.bypass,
                replica_groups=[list(range(N_CORES))],
                ins=[shard_dram[lo:hi, :]],
                outs=[full_dram[base:base + N_CORES * PART, :]],
            )
            # make Pool observe the collective once (DMA insts get 1 wait slot)
            fun = cpool.tile([1, DT], F32, tag=f"funnel{h}")
            nc.gpsimd.dma_start(out=fun[:], in_=full_dram[base:base + 1, :])

        # ---------------- layer-1 table build (id order) ----------------
        def build_table1():
            with tc.tile_pool(name="tbl", bufs=8) as tp, \
                 tc.tile_pool(name="tblp", bufs=6, space="PSUM") as pp, \
                 tc.tile_pool(name="tblbig", bufs=1) as bigp:
                big = bigp.tile([128, NT, DT], F32)
                for t in range(NT):
                    xT = tp.tile([D_IN, 128], F32, tag="xT")
                    nc.sync.dma_start(out=xT[:], in_=xT_p[:, t * 128:(t + 1) * 128])
                    psh = pp.tile([128, DT], F32, tag="psh")
                    nc.tensor.matmul(out=psh[:], lhsT=xT[:], rhs=w1_sb[:], start=True, stop=True)
                    nc.vector.tensor_copy(out=big[:, t, :], in_=psh[:])
                    nc.vector.tensor_copy(out=big[:, t, 66:67], in_=oc_sb[:, t:t + 1])
                    if (t + 1) % TPP == 0:
                        h = (t + 1) // TPP - 1
                        lo, hi = h * TPP, (h + 1) * TPP
                        nc.sync.dma_start(
                            out=tab_shard[0][lo * 128:hi * 128, :].rearrange(
                                "(t p) d -> p t d", p=128),
                            in_=big[:, lo:hi, :],
                        )
                        allgather_part(tab_shard[0], tab_full[0], h)

        # ---------------- edge phase for one layer ----------------
        def edge_phase(layer, full_dram, bias_sb, relu, dst_dram, big2):
            with tc.tile_pool(name="edg", bufs=3) as ep, \
                 tc.tile_pool(name="edg2", bufs=3) as ep2, \
                 tc.tile_pool(name="edgp", bufs=3, space="PSUM") as pp:
                moff = 0
                tglob = 0
                half_state = [0]
                pending = None  # deferred scatters: (meta_t, o3, W, G)

                def flush_pending():
                    if pending is None:
                        return
                    pm, po3, pW, pG = pending
                    for gi in range(pG):
                        nc.gpsimd.indirect_dma_start(
                            out=dst_dram[:],
                            out_offset=bass.IndirectOffsetOnAxis(
                                ap=pm[:, 2 * pW + gi:2 * pW + gi + 1], axis=0
                            ),
                            in_=po3[:, gi, :],
                            in_offset=None,
                            bounds_check=bc_reg,
                            oob_is_err=False,
                        )

                for Ks_g, W in groups:
                    G = len(Ks_g)
                    mw = 2 * W + G
                    meta_t = ep.tile([128, mw], I32, tag="meta")
                    nc.sync.dma_start(
                        out=meta_t[:],
                        in_=meta_p[moff:moff + 128 * mw].rearrange("(p w) -> p w", p=128),
                    )
                    moff += 128 * mw
                    mcol = 0 if layer == 1 else W

                    gat = ep.tile([128, W * DT], F32, tag="gat")
                    g3 = gat[:].rearrange("p (w d) -> p w d", d=DT)
                    for w in range(W):
                        nc.gpsimd.indirect_dma_start(
                            out=g3[:, w, :],
                            out_offset=None,
                            in_=full_dram[:],
                            in_offset=bass.IndirectOffsetOnAxis(
                                ap=meta_t[:, mcol + w:mcol + w + 1], axis=0
                            ),
                        )
                    # previous group's scatters issue here so the pool engine
                    # never stalls waiting for this group's DVE chain
                    flush_pending()
                    pending = None

                    # slot 0 of each tile is the self-loop row: col 65 = s_dst
                    lg = ep.tile([128, W], F32, tag="lg")
                    o = 0
                    for gi in range(G):
                        K = int(Ks_g[gi])
                        nc.vector.tensor_tensor(
                            out=lg[:, o:o + K],
                            in0=g3[:, o:o + K, 64],
                            in1=g3[:, o, 65:66].to_broadcast([128, K]),
                            op=mybir.AluOpType.add,
                        )
                        o += K
                    # e = exp(leaky_relu(lg)) = max(exp(lg), exp(0.2*lg))
                    e1 = ep.tile([128, W], F32, tag="e1")
                    nc.scalar.activation(out=e1[:], in_=lg[:], func=mybir.ActivationFunctionType.Exp)
                    e2 = ep.tile([128, W], F32, tag="e2")
                    nc.scalar.activation(out=e2[:], in_=lg[:], func=mybir.ActivationFunctionType.Exp, scale=NEG_SLOPE)
                    nc.vector.tensor_tensor(out=e1[:], in0=e1[:], in1=e2[:], op=mybir.AluOpType.max)

                    nc.vector.tensor_tensor(
                        out=g3[:, :, :],
                        in0=g3[:, :, :],
                        in1=e1[:].to_broadcast([128, W, DT]),
                        op=mybir.AluOpType.mult,
                    )

                    red = ep2.tile([128, G * DT], F32, tag="red")
                    r3 = red[:].rearrange("p (g d) -> p g d", d=DT)
                    o = 0
                    for gi in range(G):
                        K = int(Ks_g[gi])
                        nc.vector.tensor_reduce(
                            out=r3[:, gi, :],
                            in_=g3[:, o:o + K, :].rearrange("p k d -> p d k"),
                            axis=mybir.AxisListType.X,
                            op=mybir.AluOpType.add,
                        )
                        o += K
                    # epsilon so zero-degree pad rows give 0 * huge = 0, not NaN
                    den = ep2.tile([128, G], F32, tag="den")
                    nc.vector.tensor_scalar_add(den[:], r3[:, :, 66], 1e-16)
                    rec = ep2.tile([128, G], F32, tag="rec")
                    nc.vector.reciprocal(out=rec[:], in_=den[:])
                    outt = ep2.tile([128, G * 64], F32, tag="outt")
                    o3 = outt[:].rearrange("p (g d) -> p g d", d=64)
                    nc.vector.tensor_tensor(
                        out=o3[:, :, :],
                        in0=r3[:, :, 0:64],
                        in1=rec[:].to_broadcast([128, G, 64]),
                        op=mybir.AluOpType.mult,
                    )
                    for gi in range(G):
                        nc.vector.tensor_tensor(
                            out=o3[:, gi, :], in0=o3[:, gi, :], in1=bias_sb[:],
                            op=mybir.AluOpType.add,
                        )
                    if relu:
                        nc.vector.tensor_scalar_max(outt[:], outt[:], 0.0)

                    pending = (meta_t, o3, W, G)

                    if big2 is not None:
                        # fused layer-2 table build, rank order (affine)
                        for gi in range(G):
                            t = tglob + gi
                            o3m = ep2.tile([128, 64], F32, tag="o3m")
                            nc.vector.tensor_tensor(
                                out=o3m[:], in0=o3[:, gi, :],
                                in1=ocp_sb[:, t:t + 1].to_broadcast([128, 64]),
                                op=mybir.AluOpType.mult,
                            )
                            pse = pp.tile([64, 128], F32, tag="pse")
                            nc.tensor.transpose(out=pse[:], in_=o3m[:], identity=ident[:])
                            eT = ep2.tile([64, 128], F32, tag="eT")
                            nc.vector.tensor_copy(out=eT[:], in_=pse[:])
                            ps2 = pp.tile([128, DT], F32, tag="ps2")
                            nc.tensor.matmul(out=ps2[:], lhsT=eT[:], rhs=w2_sb[:], start=True, stop=True)
                            nc.vector.tensor_copy(out=big2[:, t, :], in_=ps2[:])
                            nc.vector.tensor_copy(out=big2[:, t, 66:67], in_=ocp_sb[:, t:t + 1])
                        done = tglob + G
                        while (half_state[0] + 1) * TPP <= done:
                            h = half_state[0]
                            lo, hi = h * TPP, (h + 1) * TPP
                            nc.sync.dma_start(
                                out=tab_shard[1][lo * 128:hi * 128, :].rearrange(
                                    "(t p) d -> p t d", p=128),
                                in_=big2[:, lo:hi, :],
                            )
                            allgather_part(tab_shard[1], tab_full[1], h)
                            half_state[0] += 1
                    tglob += G
                flush_pending()

        build_table1()
        with tc.tile_pool(name="big2", bufs=1) as big2p:
            big2 = big2p.tile([128, NT, DT], F32)
            edge_phase(1, tab_full[0], b1_sb, True, emb_p, big2)
            edge_phase(2, tab_full[1], b2_sb, False, out_p, None)

    _legalize_waits(nc)
    return nc


def _ensure_ntff_hook():
    """Best-effort: register the NTFF profile hook that bass_utils expects
    under axon (this agent image's antenv lacks axon_hooks)."""
    try:
        from antenv.axon_hooks import get_axon_ntff_profile_hook  # noqa: F401
        return True
    except ImportError:
        pass
    try:
        import sys
        import types
        import antenv
        from trn_agent_boot.trn_boot import _ntff_profile_via_ctypes
        hook = _ntff_profile_via_ctypes("/opt/axon/libaxon_pjrt.so")
        m = types.ModuleType("antenv.axon_hooks")
        m.get_axon_ntff_profile_hook = lambda: hook
        m.set_axon_ntff_profile_hook = lambda h: None
        sys.modules["antenv.axon_hooks"] = m
        antenv.axon_hooks = m
        return True
    except Exception:
        return False


# ----------------------------------------------------------------- driver
def kernel(x, edge_index, W1, a_src1, a_dst1, b1, W2, a_src2, a_dst2, b2):
    x = np.asarray(x, dtype=np.float32)
    W1 = np.asarray(W1, dtype=np.float32)
    W2 = np.asarray(W2, dtype=np.float32)
    a_src1 = np.asarray(a_src1, dtype=np.float32)
    a_dst1 = np.asarray(a_dst1, dtype=np.float32)
    a_src2 = np.asarray(a_src2, dtype=np.float32)
    a_dst2 = np.asarray(a_dst2, dtype=np.float32)
    b1 = np.asarray(b1, dtype=np.float32)
    b2 = np.asarray(b2, dtype=np.float32)

    metas, groups = _build_metadata(edge_index)
    nc = _build_program(groups, len(metas[0]))

    w1aug = np.zeros((D_IN, DT), dtype=np.float32)
    w1aug[:, :D_HID] = W1
    w1aug[:, 64] = W1 @ a_src1
    w1aug[:, 65] = W1 @ a_dst1
    w2aug = np.zeros((D_HID, DT), dtype=np.float32)
    w2aug[:, :D_OUT] = W2
    w2aug[:, 64] = W2 @ a_src2
    w2aug[:, 65] = W2 @ a_dst2

    x_pad = np.zeros((NPAD, D_IN), dtype=np.float32)
    x_pad[:N] = x

    # recompute the degree-sort perms to build the rank-ordered one-mask
    ei = np.asarray(edge_index)
    src = np.concatenate([np.arange(N, dtype=np.int64), ei[0]]).astype(np.int64)
    dst = np.concatenate([np.arange(N, dtype=np.int64), ei[1]]).astype(np.int64)

    in_maps = []
    for c in range(N_CORES):
        oc = np.zeros(SHARD, dtype=np.float32)
        n_real = min(max(N - c * SHARD, 0), SHARD)
        oc[:n_real] = 1.0
        lo, hi = c * SHARD, (c + 1) * SHARD
        m = (dst >= lo) & (dst < hi)
        deg = np.bincount(dst[m] - lo, minlength=SHARD)
        perm = np.argsort(-deg, kind="stable")
        ocp = oc[perm]
        in_maps.append({
            "xT": np.ascontiguousarray(x_pad[c * SHARD:(c + 1) * SHARD].T),
            "meta": metas[c],
            "w1aug": w1aug,
            "w2aug": w2aug,
            "b1": np.tile(b1, (128, 1)),
            "b2": np.tile(b2, (128, 1)),
            "onecol": oc,
            "onecolp": ocp,
        })

    trace = bool(os.environ.get("GAT_KERNEL_TRACE"))
    if trace:
        trace = _ensure_ntff_hook()
    res = run_bass_kernel_spmd(nc, in_maps, list(range(N_CORES)), trace=trace)
    if trace and res.exec_time_ns is not None:
        print(f"HW exec time: {res.exec_time_ns} ns")

    emb = np.concatenate([res.results[c]["emb"] for c in range(N_CORES)], axis=0)[:N]
    out = np.concatenate([res.results[c]["out"] for c in range(N_CORES)], axis=0)[:N]
    return (emb, out)


# revision 40
# speedup vs baseline: 1.0234x; 1.0176x over previous
"""Two-layer single-head GAT on 8 TRN2 NeuronCores (Bass/Tile, SPMD).

Strategy (graph/data parallel, dst-sharded):
  - Nodes are sharded contiguously across the 8 cores (12544 per core, 100352
    padded total). Each core owns the attention aggregation for its dst nodes.
  - Per layer, each core computes its shard of an augmented node table
      row(v) = [h(v) (64), h@a_src, h@a_dst, is_real(v), 0] (68 f32, 272B)
    with one PE matmul per 128 nodes, then shards are AllGathered (in two
    halves, overlapped with the shard build) so every core holds the full
    table in DRAM.
  - Edge phase: the core's dst nodes are sorted by degree and tiled 128 nodes
    per tile (dst node = partition, K_t edge slots along free dim; K_t is the
    cross-core max so the SPMD program is identical on all cores). Src rows
    are fetched with one indirect DMA per slot column (128 rows each); the
    self-loop edge is forced into slot 0 of every dst so the gathered row
    doubles as the source of s_dst (no separate per-tile s_dst gather). Pad
    slots point at the all-zero phantom row and contribute nothing.
      e = exp(leaky_relu(s_src + s_dst)) = max(exp(x), exp(0.2 x))
    Gathered rows are scaled by e and segment-reduced with one strided DVE
    reduction per tile; column 66 of the reduction is the softmax denominator.
    Rows are normalized, biased, (relu'd) and scattered to the output shard by
    local row id (pad nodes carry an out-of-bounds id and are dropped).
  - The layer-2 node table is built INSIDE edge phase 1: each tile's output
    rows (in degree-rank order) are transposed on the PE and projected through
    W2aug immediately, stored affine in rank order. Layer-2 gather metadata is
    rank-indexed so no scatter is needed for the table; the phantom row is the
    last rank of core 7 (a zero-degree pad node, masked to zero).

All metadata (gather/scatter indices) is precomputed on the host from
edge_index only and fed as an int32 input, so one compiled SPMD program
serves all cores.
"""
import os
import numpy as np
from contextlib import ExitStack

import concourse.bass as bass
import concourse.tile as tile
from concourse import mybir
from concourse.bass_utils import run_bass_kernel_spmd
from concourse.masks import make_identity
from concourse.vector_clock import ScopedClock

# ---------------------------------------------------------------- constants
N = 100_000
D_IN, D_HID, D_OUT = 128, 64, 64
NEG_SLOPE = 0.2
N_CORES = 8
SHARD = 12_544            # 98 * 128
NPAD = SHARD * N_CORES    # 100352
NT = SHARD // 128         # 98 node tiles per core
DT = 68                   # table row width (64 feat, s_src, s_dst, one, pad)
PHANTOM = NPAD - 1        # guaranteed all-zero table row (both layers)
OOB_ROW = 1 << 29
WCAP = 96                 # max edge slots (sum of K_t) per gather group
GCAP = 12                 # max tiles per group

F32 = mybir.dt.float32
BF16 = mybir.dt.bfloat16
I32 = mybir.dt.int32

_noop_ctr = [0]


# ------------------------------------------------------- tile-drain patch
def _patched_drain_and_barrier(self, tick_clock, wait_clock):
    """walrus codegen in this container refuses any sem wait on the Drain
    instruction; emit the kernel-tail waits as standalone single-wait NOPs
    on the sync engine instead, then an untainted drain."""
    probe = self.nc.sync.nop(nofuse=True)
    wait_clock.add_sem_waits(probe.ins, ScopedClock({None: tick_clock.global_clock}))
    si = probe.ins.sync_info
    waits = list(si.on_wait) if si is not None and si.on_wait else []
    if si is not None:
        probe.ins.sync_info = mybir.SyncInfo(
            on_wait=waits[:1], on_update=list(si.on_update or [])
        )
    for w in waits[1:]:
        nop = self.nc.sync.nop(nofuse=True)
        nop.ins.sync_info = mybir.SyncInfo(on_wait=[w], on_update=[])
    self.nc.sync.drain()
    self.nc.all_engine_barrier()
    popped = self.nc._tile_sem_poison_stack.pop()
    assert popped is self._sem_poison
    self.nc.clear_and_free_semaphores(list(self.sems.allocated().values()))
    self.nc.all_engine_barrier()


tile.TileContext._drain_and_barrier = _patched_drain_and_barrier


def _legalize_waits(nc, keep=1):
    """walrus codegen allows very few sem waits per ISA instruction (1 for
    DMAs/matmuls, 0 for Drain). Hoist excess waits onto single-wait NoOps
    on the same engine immediately before the instruction — engine program
    order preserves the blocking semantics exactly."""
    for bb in nc.main_func.blocks:
        insts = bb.instructions
        new_list = []
        changed = False
        for ins in insts:
            si = getattr(ins, "sync_info", None)
            waits = list(si.on_wait) if si is not None and si.on_wait else []
            k = 0 if ins.opcode in ("Drain",) else keep
            if len(waits) > k:
                changed = True
                for w in waits[k:]:
                    nop = mybir.InstNoOp(name=f"waitnop-{_noop_ctr[0]}", ins=[], outs=[])
                    _noop_ctr[0] += 1
                    nop.engine = ins.engine
                    nop.sync_info = mybir.SyncInfo(on_wait=[w], on_update=[])
                    new_list.append(nop)
                ins.sync_info = mybir.SyncInfo(
                    on_wait=waits[:k], on_update=list(si.on_update or [])
                )
            new_list.append(ins)
        if changed:
            insts[:] = new_list


# ------------------------------------------------------------- host prep
def _build_metadata(edge_index):
    """Host-side graph preprocessing from edge_index only.

    Self-loops are prepended so each dst's slot 0 is its own table row (the
    source of s_dst). Layer 1 gathers by global node id; layer 2 gathers by
    global RANK (the degree-sort order the layer-2 table is stored in).

    Returns (metas, groups): metas[c] is the flat int32 metadata (identical
    length per core); groups is the uniform group structure [(Ks, W), ...].
    Per group the metadata block is [128, 2*W + G] int32:
      cols [0, W)        layer-1 gather rows (global ids, PHANTOM for pads)
      cols [W, 2W)       layer-2 gather rows (global ranks, PHANTOM for pads)
      cols [2W, 2W+G)    scatter rows (local out row, OOB_ROW for pad nodes)
    """
    ei = np.asarray(edge_index)
    # self-loops FIRST so the stable dst-sort puts them at slot 0 of each dst
    src = np.concatenate([np.arange(N, dtype=np.int64), ei[0]]).astype(np.int64)
    dst = np.concatenate([np.arange(N, dtype=np.int64), ei[1]]).astype(np.int64)
    PART = SHARD // 2

    def _grow(v):
        """node/rank id -> row in the part-major full-table layout."""
        c, r = v // SHARD, v % SHARD
        return (r // PART) * (N_CORES * PART) + c * PART + (r % PART)

    per_core = []
    for c in range(N_CORES):
        lo, hi = c * SHARD, (c + 1) * SHARD
        m = (dst >= lo) & (dst < hi)
        s_c, d_c = src[m], dst[m] - lo
        order = np.argsort(d_c, kind="stable")
        s_c, d_c = s_c[order], d_c[order]
        deg = np.bincount(d_c, minlength=SHARD)
        starts = np.concatenate([[0], np.cumsum(deg)[:-1]])
        perm = np.argsort(-deg, kind="stable")
        # global rank of each global node id (for layer-2 table addressing)
        per_core.append((s_c, deg, starts, perm))

    # rank lookup: grank[global id] = core*SHARD + rank within core
    grank = np.empty(NPAD, dtype=np.int64)
    for c in range(N_CORES):
        _, _, _, perm = per_core[c]
        rank = np.empty(SHARD, dtype=np.int64)
        rank[perm] = np.arange(SHARD)
        grank[c * SHARD:(c + 1) * SHARD] = c * SHARD + rank

    Ks = np.zeros(NT, dtype=np.int64)
    for c in range(N_CORES):
        _, deg, _, perm = per_core[c]
        Ks = np.maximum(Ks, deg[perm].reshape(NT, 128).max(axis=1))
    Ks = np.maximum(Ks, 1)

    groups = []
    cur, curW = [], 0
    for t in range(NT):
        if cur and (curW + Ks[t] > WCAP or len(cur) >= GCAP):
            groups.append(cur)
            cur, curW = [], 0
        cur.append(t)
        curW += Ks[t]
    if cur:
        groups.append(cur)

    metas = []
    for c in range(N_CORES):
        s_c, deg, starts, perm = per_core[c]
        blocks = []
        for g in groups:
            cols1, cols2 = [], []
            outr = np.empty((128, len(g)), dtype=np.int64)
            for gi, t in enumerate(g):
                K = int(Ks[t])
                vs = perm[t * 128:(t + 1) * 128]
                dv = deg[vs]
                st = starts[vs]
                ar = np.arange(K)[None, :]
                valid = ar < dv[:, None]
                pos = np.minimum(st[:, None] + ar, max(len(s_c) - 1, 0))
                gsrc = np.where(valid, s_c[pos] if len(s_c) else PHANTOM, PHANTOM)
                cols1.append(_grow(gsrc))
                cols2.append(_grow(np.where(valid, grank[gsrc], PHANTOM)))
                real = dv > 0
                outr[:, gi] = np.where(real, vs, OOB_ROW)
            block = np.concatenate(cols1 + cols2 + [outr], axis=1)
            blocks.append(block.ravel())
        metas.append(np.concatenate(blocks).astype(np.int32))

    return metas, [(np.array([int(Ks[t]) for t in g]), int(sum(Ks[t] for t in g)))
                   for g in groups]


# ---------------------------------------------------------- device program
def _build_program(groups, meta_len):
    nc = bass.Bass(num_devices=N_CORES)

    xT_p = nc.declare_dram_parameter("xT", [D_IN, SHARD], F32, isOutput=False)
    meta_p = nc.declare_dram_parameter("meta", [meta_len], I32, isOutput=False)
    w1_p = nc.declare_dram_parameter("w1aug", [D_IN, DT], F32, isOutput=False)
    w2_p = nc.declare_dram_parameter("w2aug", [D_HID, DT], F32, isOutput=False)
    b1_p = nc.declare_dram_parameter("b1", [128, 64], F32, isOutput=False)
    b2_p = nc.declare_dram_parameter("b2", [128, 64], F32, isOutput=False)
    oc_p = nc.declare_dram_parameter("onecol", [SHARD], F32, isOutput=False)
    ocp_p = nc.declare_dram_parameter("onecolp", [SHARD], F32, isOutput=False)
    emb_p = nc.declare_dram_parameter("emb", [SHARD, D_HID], F32, isOutput=True)
    out_p = nc.declare_dram_parameter("out", [SHARD, D_OUT], F32, isOutput=True)

    tab_shard = [nc.dram_tensor(f"tab{l}_shard", [SHARD, DT], BF16) for l in (1, 2)]
    tab_full = [nc.dram_tensor(f"tab{l}_full", [NPAD, DT], BF16) for l in (1, 2)]

    NPART = 2                    # AllGather pipeline depth (divides NT)
    PART = SHARD // NPART        # rows per part
    TPP = NT // NPART            # tiles per part

    with tile.TileContext(nc) as tc, ExitStack() as ctx:
        cpool = ctx.enter_context(tc.tile_pool(name="const", bufs=1))
        ident = cpool.tile([128, 128], F32)
        make_identity(nc, ident[:])
        w1_sb = cpool.tile([D_IN, DT], F32)
        nc.sync.dma_start(out=w1_sb[:], in_=w1_p[:])
        w2_sb = cpool.tile([D_HID, DT], F32)
        nc.sync.dma_start(out=w2_sb[:], in_=w2_p[:])
        oc_sb = cpool.tile([128, NT], F32)
        nc.sync.dma_start(out=oc_sb[:], in_=oc_p[:].rearrange("(t p) -> p t", p=128))
        ocp_sb = cpool.tile([128, NT], F32)
        nc.sync.dma_start(out=ocp_sb[:], in_=ocp_p[:].rearrange("(t p) -> p t", p=128))
        b1_sb = cpool.tile([128, 64], F32)
        nc.sync.dma_start(out=b1_sb[:], in_=b1_p[:])
        b2_sb = cpool.tile([128, 64], F32)
        nc.sync.dma_start(out=b2_sb[:], in_=b2_p[:])

        bc_reg = nc.gpsimd.alloc_register()
        nc.gpsimd.reg_mov(bc_reg, SHARD - 1)

        def allgather_part(shard_dram, full_dram, h):
            # full table is laid out part-major: rows [h*8*PART, (h+1)*8*PART)
            # hold part h of every core's shard, concatenated by core — so the
            # collective output is contiguous. Host gather indices use the
            # same layout (see _grow).
            lo, hi = h * PART, (h + 1) * PART
            base = h * N_CORES * PART
            nc.gpsimd.collective_compute(
                "AllGather",
                mybir.AluOpType.bypass,
                replica_groups=[list(range(N_CORES))],
                ins=[shard_dram[lo:hi, :]],
                outs=[full_dram[base:base + N_CORES * PART, :]],
            )
            # make Pool observe the collective once (DMA insts get 1 wait slot)
            fun = cpool.tile([1, DT], BF16, tag=f"funnel{h}")
            nc.gpsimd.dma_start(out=fun[:], in_=full_dram[base:base + 1, :])

        # ---------------- layer-1 table build (id order) ----------------
        def build_table1():
            with tc.tile_pool(name="tbl", bufs=8) as tp, \
                 tc.tile_pool(name="tblp", bufs=6, space="PSUM") as pp, \
                 tc.tile_pool(name="tblbig", bufs=1) as bigp:
                big = bigp.tile([128, NT, DT], BF16)
                for t in range(NT):
                    xT = tp.tile([D_IN, 128], F32, tag="xT")
                    nc.sync.dma_start(out=xT[:], in_=xT_p[:, t * 128:(t + 1) * 128])
                    psh = pp.tile([128, DT], F32, tag="psh")
                    nc.tensor.matmul(out=psh[:], lhsT=xT[:], rhs=w1_sb[:], start=True, stop=True)
                    nc.vector.tensor_copy(out=big[:, t, :], in_=psh[:])
                    nc.vector.tensor_copy(out=big[:, t, 66:67], in_=oc_sb[:, t:t + 1])
                    if (t + 1) % TPP == 0:
                        h = (t + 1) // TPP - 1
                        lo, hi = h * TPP, (h + 1) * TPP
                        nc.sync.dma_start(
                            out=tab_shard[0][lo * 128:hi * 128, :].rearrange(
                                "(t p) d -> p t d", p=128),
                            in_=big[:, lo:hi, :],
                        )
                        allgather_part(tab_shard[0], tab_full[0], h)

        # ---------------- edge phase for one layer ----------------
        def edge_phase(layer, full_dram, bias_sb, relu, dst_dram, big2):
            with tc.tile_pool(name="edg", bufs=3) as ep, \
                 tc.tile_pool(name="edg2", bufs=3) as ep2, \
                 tc.tile_pool(name="edgp", bufs=3, space="PSUM") as pp:
                moff = 0
                tglob = 0
                half_state = [0]
                pending = None  # deferred scatters: (meta_t, o3, W, G)

                def flush_pending():
                    if pending is None:
                        return
                    pm, po3, pW, pG = pending
                    for gi in range(pG):
                        nc.gpsimd.indirect_dma_start(
                            out=dst_dram[:],
                            out_offset=bass.IndirectOffsetOnAxis(
                                ap=pm[:, 2 * pW + gi:2 * pW + gi + 1], axis=0
                            ),
                            in_=po3[:, gi, :],
                            in_offset=None,
                            bounds_check=bc_reg,
                            oob_is_err=False,
                        )

                for Ks_g, W in groups:
                    G = len(Ks_g)
                    mw = 2 * W + G
                    meta_t = ep.tile([128, mw], I32, tag="meta")
                    nc.sync.dma_start(
                        out=meta_t[:],
                        in_=meta_p[moff:moff + 128 * mw].rearrange("(p w) -> p w", p=128),
                    )
                    moff += 128 * mw
                    mcol = 0 if layer == 1 else W

                    gat = ep.tile([128, W * DT], BF16, tag="gat")
                    g3 = gat[:].rearrange("p (w d) -> p w d", d=DT)
                    for w in range(W):
                        nc.gpsimd.indirect_dma_start(
                            out=g3[:, w, :],
                            out_offset=None,
                            in_=full_dram[:],
                            in_offset=bass.IndirectOffsetOnAxis(
                                ap=meta_t[:, mcol + w:mcol + w + 1], axis=0
                            ),
                        )
                    # previous group's scatters issue here so the pool engine
                    # never stalls waiting for this group's DVE chain
                    flush_pending()
                    pending = None

                    # slot 0 of each tile is the self-loop row: col 65 = s_dst
                    lg = ep.tile([128, W], F32, tag="lg")
                    o = 0
                    for gi in range(G):
                        K = int(Ks_g[gi])
                        nc.vector.tensor_tensor(
                            out=lg[:, o:o + K],
                            in0=g3[:, o:o + K, 64],
                            in1=g3[:, o, 65:66].to_broadcast([128, K]),
                            op=mybir.AluOpType.add,
                        )
                        o += K
                    # e = exp(leaky_relu(lg)) = max(exp(lg), exp(0.2*lg))
                    e1 = ep.tile([128, W], BF16, tag="e1")
                    nc.scalar.activation(out=e1[:], in_=lg[:], func=mybir.ActivationFunctionType.Exp)
                    e2 = ep.tile([128, W], BF16, tag="e2")
                    nc.scalar.activation(out=e2[:], in_=lg[:], func=mybir.ActivationFunctionType.Exp, scale=NEG_SLOPE)
                    nc.vector.tensor_tensor(out=e1[:], in0=e1[:], in1=e2[:], op=mybir.AluOpType.max)

                    nc.vector.tensor_tensor(
                        out=g3[:, :, :],
                        in0=g3[:, :, :],
                        in1=e1[:].to_broadcast([128, W, DT]),
                        op=mybir.AluOpType.mult,
                    )

                    red = ep2.tile([128, G * DT], F32, tag="red")
                    r3 = red[:].rearrange("p (g d) -> p g d", d=DT)
                    o = 0
                    for gi in range(G):
                        K = int(Ks_g[gi])
                        nc.vector.tensor_reduce(
                            out=r3[:, gi, :],
                            in_=g3[:, o:o + K, :].rearrange("p k d -> p d k"),
                            axis=mybir.AxisListType.X,
                            op=mybir.AluOpType.add,
                        )
                        o += K
                    # epsilon so zero-degree pad rows give 0 * huge = 0, not NaN
                    den = ep2.tile([128, G], F32, tag="den")
                    nc.vector.tensor_scalar_add(den[:], r3[:, :, 66], 1e-16)
                    rec = ep2.tile([128, G], F32, tag="rec")
                    nc.vector.reciprocal(out=rec[:], in_=den[:])
                    outt = ep2.tile([128, G * 64], F32, tag="outt")
                    o3 = outt[:].rearrange("p (g d) -> p g d", d=64)
                    nc.vector.tensor_tensor(
                        out=o3[:, :, :],
                        in0=r3[:, :, 0:64],
                        in1=rec[:].to_broadcast([128, G, 64]),
                        op=mybir.AluOpType.mult,
                    )
                    for gi in range(G):
                        nc.vector.tensor_tensor(
                            out=o3[:, gi, :], in0=o3[:, gi, :], in1=bias_sb[:],
                            op=mybir.AluOpType.add,
                        )
                    if relu:
                        nc.vector.tensor_scalar_max(outt[:], outt[:], 0.0)

                    pending = (meta_t, o3, W, G)

                    if big2 is not None:
                        # fused layer-2 table build, rank order (affine)
                        for gi in range(G):
                            t = tglob + gi
                            o3m = ep2.tile([128, 64], F32, tag="o3m")
                            nc.vector.tensor_tensor(
                                out=o3m[:], in0=o3[:, gi, :],
                                in1=ocp_sb[:, t:t + 1].to_broadcast([128, 64]),
                                op=mybir.AluOpType.mult,
                            )
                            pse = pp.tile([64, 128], F32, tag="pse")
                            nc.tensor.transpose(out=pse[:], in_=o3m[:], identity=ident[:])
                            eT = ep2.tile([64, 128], F32, tag="eT")
                            nc.vector.tensor_copy(out=eT[:], in_=pse[:])
                            # (big2 is bf16; ps2 f32 -> bf16 on copy below)
                            ps2 = pp.tile([128, DT], F32, tag="ps2")
                            nc.tensor.matmul(out=ps2[:], lhsT=eT[:], rhs=w2_sb[:], start=True, stop=True)
                            nc.vector.tensor_copy(out=big2[:, t, :], in_=ps2[:])
                            nc.vector.tensor_copy(out=big2[:, t, 66:67], in_=ocp_sb[:, t:t + 1])
                        done = tglob + G
                        while (half_state[0] + 1) * TPP <= done:
                            h = half_state[0]
                            lo, hi = h * TPP, (h + 1) * TPP
                            nc.sync.dma_start(
                                out=tab_shard[1][lo * 128:hi * 128, :].rearrange(
                                    "(t p) d -> p t d", p=128),
                                in_=big2[:, lo:hi, :],
                            )
                            allgather_part(tab_shard[1], tab_full[1], h)
                            half_state[0] += 1
                    tglob += G
                flush_pending()

        build_table1()
        with tc.tile_pool(name="big2", bufs=1) as big2p:
            big2 = big2p.tile([128, NT, DT], BF16)
            edge_phase(1, tab_full[0], b1_sb, True, emb_p, big2)
            edge_phase(2, tab_full[1], b2_sb, False, out_p, None)

    _legalize_waits(nc)
    return nc


def _ensure_ntff_hook():
    """Best-effort: register the NTFF profile hook that bass_utils expects
    under axon (this agent image's antenv lacks axon_hooks)."""
    try:
        from antenv.axon_hooks import get_axon_ntff_profile_hook  # noqa: F401
        return True
    except ImportError:
        pass
    try:
        import sys
        import types
        import antenv
        from trn_agent_boot.trn_boot import _ntff_profile_via_ctypes
        hook = _ntff_profile_via_ctypes("/opt/axon/libaxon_pjrt.so")
        m = types.ModuleType("antenv.axon_hooks")
        m.get_axon_ntff_profile_hook = lambda: hook
        m.set_axon_ntff_profile_hook = lambda h: None
        sys.modules["antenv.axon_hooks"] = m
        antenv.axon_hooks = m
        return True
    except Exception:
        return False


# ----------------------------------------------------------------- driver
def kernel(x, edge_index, W1, a_src1, a_dst1, b1, W2, a_src2, a_dst2, b2):
    x = np.asarray(x, dtype=np.float32)
    W1 = np.asarray(W1, dtype=np.float32)
    W2 = np.asarray(W2, dtype=np.float32)
    a_src1 = np.asarray(a_src1, dtype=np.float32)
    a_dst1 = np.asarray(a_dst1, dtype=np.float32)
    a_src2 = np.asarray(a_src2, dtype=np.float32)
    a_dst2 = np.asarray(a_dst2, dtype=np.float32)
    b1 = np.asarray(b1, dtype=np.float32)
    b2 = np.asarray(b2, dtype=np.float32)

    metas, groups = _build_metadata(edge_index)
    nc = _build_program(groups, len(metas[0]))

    w1aug = np.zeros((D_IN, DT), dtype=np.float32)
    w1aug[:, :D_HID] = W1
    w1aug[:, 64] = W1 @ a_src1
    w1aug[:, 65] = W1 @ a_dst1
    w2aug = np.zeros((D_HID, DT), dtype=np.float32)
    w2aug[:, :D_OUT] = W2
    w2aug[:, 64] = W2 @ a_src2
    w2aug[:, 65] = W2 @ a_dst2

    x_pad = np.zeros((NPAD, D_IN), dtype=np.float32)
    x_pad[:N] = x

    # recompute the degree-sort perms to build the rank-ordered one-mask
    ei = np.asarray(edge_index)
    src = np.concatenate([np.arange(N, dtype=np.int64), ei[0]]).astype(np.int64)
    dst = np.concatenate([np.arange(N, dtype=np.int64), ei[1]]).astype(np.int64)

    in_maps = []
    for c in range(N_CORES):
        oc = np.zeros(SHARD, dtype=np.float32)
        n_real = min(max(N - c * SHARD, 0), SHARD)
        oc[:n_real] = 1.0
        lo, hi = c * SHARD, (c + 1) * SHARD
        m = (dst >= lo) & (dst < hi)
        deg = np.bincount(dst[m] - lo, minlength=SHARD)
        perm = np.argsort(-deg, kind="stable")
        ocp = oc[perm]
        in_maps.append({
            "xT": np.ascontiguousarray(x_pad[c * SHARD:(c + 1) * SHARD].T),
            "meta": metas[c],
            "w1aug": w1aug,
            "w2aug": w2aug,
            "b1": np.tile(b1, (128, 1)),
            "b2": np.tile(b2, (128, 1)),
            "onecol": oc,
            "onecolp": ocp,
        })

    trace = bool(os.environ.get("GAT_KERNEL_TRACE"))
    if trace:
        trace = _ensure_ntff_hook()
    res = run_bass_kernel_spmd(nc, in_maps, list(range(N_CORES)), trace=trace)
    if trace and res.exec_time_ns is not None:
        print(f"HW exec time: {res.exec_time_ns} ns")

    emb = np.concatenate([res.results[c]["emb"] for c in range(N_CORES)], axis=0)[:N]
    out = np.concatenate([res.results[c]["out"] for c in range(N_CORES)], axis=0)[:N]
    return (emb, out)
